# revision 13
# baseline (speedup 1.0000x reference)
"""Trainium2 Bass kernel for nn_DiffusionLM (dense_mlp).

Strategy (8 NeuronCores, data-parallel over tokens; 512 tokens/core):
  - Host: embedding gather + pre-transpose h0 -> h0T [HID, T_CORE] f32;
    weights cast to bf16; step-bias table r1[t] and step coefficient
    folded by the cumulative 1/sqrt(alpha) product A_t (LayerNorm is
    scale-invariant, so the per-step `isa` rescale of h is dropped and
    absorbed into r1/A_t and coef/A_t); vocab head (embed*gn).T in bf16.
  - Device diffusion (20 steps), all bf16 matmuls at 1 cycle/row:
      * h~T kept feature-major in SBUF (f32 master + bf16 copy), so the
        first-layer matmul needs no activation transpose.
      * z1/z2 LayerNorm+gelu fused into a single ACT pass per PSUM half:
        Gelu(psum*rstd + (-mu*rstd)) with per-partition scale/bias APs.
      * z1/z2 transposed back to feature-major by DMA XBAR transposes
        (SBUF->SBUF, off the PE).
      * h~ update is one gpsimd scalar_tensor_tensor: h~ += (-c~)*scoreT.
  - Final LN folded into the vocab projection: center h~T by mu (rank-0
    DMA broadcast of the mu row), fold rstd into the PSUM->SBUF evac.
  - Vocab projection streams embt bf16 from HBM (first chunks prefetched
    during diffusion), writes f32 logits.
"""

import numpy as np
import ml_dtypes

import concourse.bass as bass
import concourse.mybir as mybir
import concourse.tile as tile
from concourse import bacc, bass_utils
from concourse.bass import ds, ts
from concourse.masks import make_identity

dt = mybir.dt
F32 = dt.float32
BF16 = dt.bfloat16
I32 = dt.int32
AF = mybir.ActivationFunctionType
ALU = mybir.AluOpType

N_CORES = 8
VOCAB = 32000
HID = 512
DH = 2 * HID  # 1024
N_STEPS = 20
EPS = 1e-5
B, S = 2, 2048
T_TOTAL = B * S              # 4096
T_CORE = T_TOTAL // N_CORES  # 512
P = 128
TPN = T_CORE // P            # 4 token tiles
KH = HID // P                # 4
KD = DH // P                 # 8
RSQRT_MAGIC = 0x5F3759DF
VC = 2048                    # vocab stream chunk
N_PREF = 3                   # chunks prefetched during diffusion
EMB_BUFS = 4                 # embt chunk ring size


def _step_consts(n_steps):
    """Per-step scalars, ordered t = n_steps-1 .. 0, matching reference."""
    betas = np.linspace(0.0001, 0.02, n_steps, dtype=np.float32)
    alphas = (1.0 - betas).astype(np.float32)
    acp = np.cumprod(alphas, dtype=np.float32)
    tsx = np.arange(n_steps - 1, -1, -1)
    t_norm = (tsx.astype(np.float32) / np.float32(n_steps)).astype(np.float32)
    coef = (betas[tsx] / np.sqrt((np.float32(1.0) - acp[tsx]))).astype(np.float32)
    isa = (np.float32(1.0) / np.sqrt(alphas[tsx])).astype(np.float32)
    # A_s = prod_{j<s} isa_j; h = A*h~ and LN() erases the final A.
    A = np.ones(n_steps, dtype=np.float64)
    for s_ in range(1, n_steps):
        A[s_] = A[s_ - 1] * isa[s_ - 1]
    ctil = (coef.astype(np.float64) / A).astype(np.float32)
    return t_norm, coef, isa, A.astype(np.float32), ctil


def build_program(n_steps=N_STEPS, vocab=VOCAB,
                  apply_gb1=False, apply_gb2=False,
                  use_b2=False, use_b3=False, use_voff=False):
    nc = bacc.Bacc("TRN2", target_bir_lowering=False, debug=False,
                   num_devices=N_CORES)

    h0t_d = nc.dram_tensor("h0t", [HID, T_CORE], F32, kind="ExternalInput").ap()
    w1_d = nc.dram_tensor("w1", [HID, DH], BF16, kind="ExternalInput").ap()
    r1_d = nc.dram_tensor("r1", [1, n_steps, DH], BF16,
                          kind="ExternalInput").ap()
    w2_d = nc.dram_tensor("w2", [DH, DH], BF16, kind="ExternalInput").ap()
    w3_d = nc.dram_tensor("w3", [DH, HID], BF16, kind="ExternalInput").ap()
    emb_d = nc.dram_tensor("embt", [HID, vocab], BF16,
                           kind="ExternalInput").ap()
    out_d = nc.dram_tensor("logits", [T_CORE, vocab], BF16,
                           kind="ExternalOutput").ap()
    b2_d = b3_d = voff_d = gb_d = None
    if use_b2:
        b2_d = nc.dram_tensor("b2", [1, DH], BF16, kind="ExternalInput").ap()
    if use_b3:
        b3_d = nc.dram_tensor("b3", [1, HID], BF16, kind="ExternalInput").ap()
    if use_voff:
        voff_d = nc.dram_tensor("voff", [1, vocab], F32,
                                kind="ExternalInput").ap()
    if apply_gb1 or apply_gb2:
        gb_d = nc.dram_tensor("gb", [4, DH], F32, kind="ExternalInput").ap()

    _, _, isa_c, A_c, ctil = _step_consts(n_steps)
    eps1 = (EPS / (A_c.astype(np.float64) ** 2)).astype(np.float32)
    a_fin = float(A_c[-1] * isa_c[-1])
    eps_fin = float(EPS / (a_fin * a_fin))

    with tile.TileContext(nc) as tc:
      with (
          tc.tile_pool(name="wpool", bufs=1) as wpool,
          tc.tile_pool(name="work", bufs=3) as work,
          tc.tile_pool(name="emb", bufs=EMB_BUFS) as embp,
          tc.tile_pool(name="lout", bufs=3) as loutp,
          tc.tile_pool(name="ps", bufs=8, space="PSUM") as psp,
      ):
            # ---- resident constants / weights ----
            ones1 = wpool.tile([1, P], BF16)
            nc.vector.memset(ones1, 1.0)
            ident = wpool.tile([P, P], F32)
            make_identity(nc, ident)

            # startup loads split across both HWDGE queues (sync+scalar) so
            # the first-step matmuls aren't gated on one serial queue.
            hT = wpool.tile([P, KH, T_CORE], F32)
            h0t_r = h0t_d.rearrange("(k p) t -> p k t", p=P)
            w1_r = w1_d.rearrange("(k p) n -> p k n", p=P)
            w1a = wpool.tile([P, KH, DH], BF16)
            for kc in range(KH):
                eng = nc.sync if kc % 2 == 0 else nc.scalar
                eng.dma_start(out=hT[:, kc, :], in_=h0t_r[:, kc, :])
            for kc in range(KH):
                eng = nc.sync if kc % 2 == 1 else nc.scalar
                eng.dma_start(out=w1a[:, kc, :], in_=w1_r[:, kc, :])
            w2_r = w2_d.rearrange("(k p) n -> p k n", p=P)
            w2a = wpool.tile([P, KD, DH], BF16)
            for half in range(2):
                eng = nc.sync if half == 0 else nc.scalar
                eng.dma_start(out=w2a[:, ds(half * 4, 4), :],
                              in_=w2_r[:, ds(half * 4, 4), :])
            w3a = wpool.tile([P, KD, HID], BF16)
            nc.gpsimd.dma_start(out=w3a,
                                in_=w3_d.rearrange("(k p) n -> p k n", p=P))
            w1s = [w1a[:, kc, :] for kc in range(KH)]
            w2s = [w2a[:, kc, :] for kc in range(KD)]
            w3s = [w3a[:, kc, :] for kc in range(KD)]

            b2s = b3s = voff_s = onesrow = gbs = None
            if use_b2:
                b2s = wpool.tile([1, DH], BF16)
                nc.sync.dma_start(out=b2s, in_=b2_d)
            if use_b3:
                b3s = wpool.tile([1, HID], BF16)
                nc.sync.dma_start(out=b3s, in_=b3_d)
                onesrow = wpool.tile([1, T_CORE], BF16)
                nc.vector.memset(onesrow, 1.0)
            if use_voff:
                voff_s = wpool.tile([1, vocab], F32)
                nc.sync.dma_start(out=voff_s, in_=voff_d)
                voff_bc = wpool.tile([P, VC], F32)
            if gb_d is not None:
                gbs = wpool.tile([P, 4, DH], F32)
                nc.sync.dma_start(out=gbs, in_=gb_d.to_broadcast([P, 4, DH]))

            # persistent h~T (feature-major): f32 master + bf16 matmul copy
            # (initial casts on DVE: the ACT queue is busy loading w2/w3)
            hbf = wpool.tile([P, KH, T_CORE], BF16)
            for kc in range(KH):
                nc.vector.tensor_copy(out=hbf[:, kc, :], in_=hT[:, kc, :])
            hcT = wpool.tile([P, KH, T_CORE], BF16)
            magict = wpool.tile([P, TPN], I32)
            nc.vector.memset(magict, RSQRT_MAGIC)

            def rsqrt_chain(mvp, n, eps):
                """DVE chain on [P,n]: returns (rstd, negbias) tiles.

                mvp is [P,n,2] f32 (mean, var) from bn_aggr; eps is the
                A-rescaled epsilon keeping LN scale-exact vs the reference."""
                u = work.tile([P, n], F32, tag="u", bufs=4)
                yv = work.tile([P, n], F32, tag="yv", bufs=4)
                t2 = work.tile([P, n], F32, tag="t2", bufs=4)
                nb = work.tile([P, n], F32, tag="nb", bufs=4)
                nc.vector.tensor_scalar(out=u, in0=mvp[:, :, 1], scalar1=eps,
                                        scalar2=None, op0=ALU.add)
                nc.vector.tensor_scalar(out=t2.bitcast(I32),
                                        in0=u.bitcast(I32), scalar1=1,
                                        scalar2=None,
                                        op0=ALU.logical_shift_right)
                nc.vector.tensor_tensor(out=yv.bitcast(I32),
                                        in0=magict[:, :n],
                                        in1=t2.bitcast(I32), op=ALU.subtract)
                # Newton 1: rstd = est * (1.5 - 0.5*u*est^2), fused
                nc.vector.tensor_tensor(out=t2, in0=yv, in1=yv, op=ALU.mult)
                nc.vector.scalar_tensor_tensor(out=t2, in0=t2, scalar=-0.5,
                                               in1=u, op0=ALU.mult,
                                               op1=ALU.mult)
                nc.vector.scalar_tensor_tensor(out=yv, in0=t2, scalar=1.5,
                                               in1=yv, op0=ALU.add,
                                               op1=ALU.mult)
                # nb = -mean * rstd
                nc.vector.scalar_tensor_tensor(out=nb, in0=mvp[:, :, 0],
                                               scalar=-1.0, in1=yv,
                                               op0=ALU.mult, op1=ALU.mult)
                return yv, nb

            # LN-chain emission groups: tile 0 alone (unblocks ACT/PE
            # early), then (1,2), then 3.
            LN_GROUPS = {0: (0,), 2: (1, 2), 3: (3,)}

            def ln_gelu_group(pps, sts, group, zall, gb_idx, eps):
                """Stats-chain + fused LN/gelu into zall[:, t, :]."""
                ng = len(group)
                mvp = work.tile([P, ng, 2], F32, tag=f"mv{ng}", bufs=4)
                for i, t in enumerate(group):
                    nc.vector.bn_aggr(out=mvp[:, i, :], in_=sts[t])
                rstd, nb = rsqrt_chain(mvp, ng, eps)
                for i, t in enumerate(group):
                    if gb_idx is None:
                        for h in range(2):
                            nc.scalar.activation(
                                out=zall[:, t, ds(h * 512, 512)],
                                in_=pps[t][h], func=AF.Gelu,
                                scale=rstd[:, i:i + 1], bias=nb[:, i:i + 1])
                    else:
                        # general path: g/be per-feature after LN
                        zf = work.tile([P, DH], F32, tag="zf", bufs=2)
                        for h in range(2):
                            nc.vector.tensor_scalar(
                                out=zf[:, ds(h * 512, 512)], in0=pps[t][h],
                                scalar1=mvp[:, i, 0:1],
                                scalar2=rstd[:, i:i + 1],
                                op0=ALU.subtract, op1=ALU.mult)
                        g_t = gbs[:, gb_idx, :]
                        be_t = gbs[:, gb_idx + 1, :]
                        nc.vector.tensor_tensor(out=zf, in0=zf, in1=g_t,
                                                op=ALU.mult)
                        nc.vector.tensor_tensor(out=zf, in0=zf, in1=be_t,
                                                op=ALU.add)
                        nc.scalar.activation(out=zall[:, t, :], in_=zf,
                                             func=AF.Gelu)

            # ================= diffusion =================
            ets = []

            def load_et(vc, in_head=False):
                v0e = vc * VC
                vne = min(VC, vocab - v0e)
                et = embp.tile([P, KH, VC], BF16, tag="et",
                               name=f"et_{vc}")
                # during diffusion the HWDGE queues are busy with
                # transposes; in the head they are free, so alternate.
                eng = (nc.sync if vc % 2 == 0 else nc.gpsimd) if in_head \
                    else nc.gpsimd
                for kc in range(KH):
                    eng.dma_start(
                        out=et[:, kc, :vne],
                        in_=emb_d[kc * P:(kc + 1) * P, v0e:v0e + vne])
                ets.append(et)
            for step in range(n_steps):
                cneg = -float(ctil[step])
                r1row = work.tile([1, DH], BF16, tag="r1row", bufs=2,
                                  name=f"r1row_{step}")
                nc.gpsimd.dma_start(out=r1row, in_=r1_d[:, step, :])

                # ---- layer 1: z1 = gelu(LN(h~ @ W1 + r1~)) ----
                z1ps, z1st = {}, {}
                z1all = work.tile([P, TPN, DH], BF16, tag="z1all", bufs=2,
                                  name=f"z1all_{step}")
                z1T = work.tile([P, TPN, KD, P], BF16, tag="z1T", bufs=2,
                                name=f"z1T_{step}")
                for tp in range(TPN):
                    pp = [psp.tile([P, 512], F32, tag="ps",
                                   name=f"ps1_{step}_{tp}_{h}")
                          for h in range(2)]
                    for kc in range(KH):
                        for h in range(2):
                            nc.tensor.matmul(pp[h], hbf[:, kc, ts(tp, P)],
                                             w1s[kc][:, ds(h * 512, 512)],
                                             start=(kc == 0), stop=False)
                    for h in range(2):
                        nc.tensor.matmul(pp[h], ones1,
                                         r1row[:, ds(h * 512, 512)],
                                         start=False, stop=True)
                    st = work.tile([P, 2, 6], F32, tag="st", bufs=4)
                    for h in range(2):
                        nc.vector.bn_stats(out=st[:, h, :], in_=pp[h])
                    z1ps[tp], z1st[tp] = pp, st
                    if tp in LN_GROUPS:
                        ln_gelu_group(z1ps, z1st, LN_GROUPS[tp], z1all,
                                      0 if apply_gb1 else None,
                                      float(eps1[step]))
                    if tp in LN_GROUPS:
                        for t in LN_GROUPS[tp]:
                            for h in range(2):
                                eng = nc.sync if (t + h) % 2 == 0 else nc.scalar
                                eng.dma_start(
                                    out=z1T[:, t, ds(h * KH, KH), :],
                                    in_=z1all[:, t, ds(h * 512, 512)],
                                    transpose=True)


                # ---- layer 2: z2 = gelu(LN(z1 @ W2 (+b2))) ----
                z2ps, z2st = {}, {}
                z2all = work.tile([P, TPN, DH], BF16, tag="z2all", bufs=2,
                                  name=f"z2all_{step}")
                z2T = work.tile([P, TPN, KD, P], BF16, tag="z2T", bufs=2,
                                name=f"z2T_{step}")
                for tp in range(TPN):
                    pp = [psp.tile([P, 512], F32, tag="ps",
                                   name=f"ps2_{step}_{tp}_{h}")
                          for h in range(2)]
                    for kc in range(KD):
                        for h in range(2):
                            nc.tensor.matmul(pp[h], z1T[:, tp, kc, :],
                                             w2s[kc][:, ds(h * 512, 512)],
                                             start=(kc == 0),
                                             stop=(kc == KD - 1 and not use_b2))
                    if use_b2:
                        for h in range(2):
                            nc.tensor.matmul(pp[h], ones1,
                                             b2s[:, ds(h * 512, 512)],
                                             start=False, stop=True)
                    st = work.tile([P, 2, 6], F32, tag="st", bufs=4)
                    for h in range(2):
                        nc.vector.bn_stats(out=st[:, h, :], in_=pp[h])
                    z2ps[tp], z2st[tp] = pp, st
                    if tp in LN_GROUPS:
                        ln_gelu_group(z2ps, z2st, LN_GROUPS[tp], z2all,
                                      2 if apply_gb2 else None,
                                      float(eps1[step]))
                    if tp in LN_GROUPS:
                        for t in LN_GROUPS[tp]:
                            for h in range(2):
                                eng = nc.sync if (t + h) % 2 == 1 else nc.scalar
                                eng.dma_start(
                                    out=z2T[:, t, ds(h * KH, KH), :],
                                    in_=z2all[:, t, ds(h * 512, 512)],
                                    transpose=True)


                # ---- layer 3 (feature-major) + h~ update ----
                # hbf for the next step is produced directly from PSUM per
                # token-half so the next mm1 isn't gated on the f32 master.
                ps3 = [psp.tile([P, 512], F32, tag="ps",
                                name=f"ps3_{step}_{mc}")
                       for mc in range(KH)]
                for hn in range(2):  # token halves so PE starts earlier
                    sl = ds(hn * 256, 256)
                    for mc in range(KH):
                        for kc in range(KD):
                            nc.tensor.matmul(
                                ps3[mc][:, sl], w3s[kc][:, ts(mc, P)],
                                z2T[:, 2 * hn:2 * hn + 2, kc, :],
                                start=(kc == 0),
                                stop=(kc == KD - 1 and not use_b3))
                        if use_b3:
                            nc.tensor.matmul(ps3[mc][:, sl],
                                             b3s[:, ts(mc, P)], onesrow[:, sl],
                                             start=False, stop=True)
                    if step < n_steps - 1:
                        for mc in range(KH):
                            nc.vector.scalar_tensor_tensor(
                                out=hbf[:, mc, sl], in0=ps3[mc][:, sl],
                                scalar=cneg, in1=hT[:, mc, sl],
                                op0=ALU.mult, op1=ALU.add)
                # f32 master update off the DVE: stage score via ACT copy,
                # then gpsimd does hT += cneg*score (SBUF-only operands).
                for mc in range(KH):
                    nc.vector.scalar_tensor_tensor(
                        out=hT[:, mc, :], in0=ps3[mc], scalar=cneg,
                        in1=hT[:, mc, :], op0=ALU.mult, op1=ALU.add)

                # prefetch first embt chunks late in diffusion
                if n_steps - 1 - N_PREF <= step < n_steps - 1:
                    load_et(step - (n_steps - 1 - N_PREF))

            # ============ final LN (folded into vocab head) ============
            # PE transposes h~T into token-major PSUM tiles for stats
            # (the PE is otherwise idle between diffusion and vocab).
            pst = [psp.tile([P, 512], F32, tag="ps", name=f"pst_{ti}")
                   for ti in range(TPN)]
            mvf = wpool.tile([P, TPN, 2], F32)
            for ti in range(TPN):
                for kc in range(KH):
                    nc.tensor.transpose(pst[ti][:, ts(kc, P)],
                                        hT[:, kc, ts(ti, P)], ident)
                stf = work.tile([P, KH, 6], F32, tag="stf", bufs=4)
                for kc in range(KH):
                    nc.vector.bn_stats(out=stf[:, kc, :],
                                       in_=pst[ti][:, ts(kc, P)])
                nc.vector.bn_aggr(out=mvf[:, ti, :], in_=stf)
            rsf, _nbf = rsqrt_chain(mvf, TPN, eps_fin)
            # mu row -> [1, T_CORE] bf16 via PE transpose + one DMA, then
            # broadcast across partitions with a rank-1 PE matmul.
            ptm = psp.tile([P, 512], F32, tag="ps")
            nc.tensor.transpose(ptm[0:TPN, 0:P], mvf[:, :, 0], ident)
            mur4 = wpool.tile([P, P], BF16, name="mur4")
            nc.vector.tensor_copy(out=mur4[0:TPN, :], in_=ptm[0:TPN, 0:P])
            murow = wpool.tile([1, T_CORE], BF16, name="murow")
            nc.sync.dma_start(out=murow, in_=mur4[0:TPN, :])
            mu_bc = psp.tile([P, 512], F32, tag="ps", name="mu_bc")
            nc.tensor.matmul(mu_bc, ones1, murow, start=True, stop=True)
            for kc in range(KH):
                nc.vector.tensor_tensor(out=hcT[:, kc, :], in0=hT[:, kc, :],
                                        in1=mu_bc, op=ALU.subtract)

            # ================= vocab head =================
            n_vc = (vocab + VC - 1) // VC

            def evac_logits(lo_sl, pl_sl, tp):
                # ACT engine: lo = rstd * psum (scalar queue; frees DVE and
                # rounds f32->bf16 properly)
                nc.scalar.activation(out=lo_sl, in_=pl_sl, func=AF.Copy,
                                     scale=rsf[:, tp:tp + 1])

            for vc in range(len(ets), min(EMB_BUFS, n_vc)):
                load_et(vc, in_head=True)
            for vc in range(n_vc):
                v0 = vc * VC
                vn = min(VC, vocab - v0)
                et = ets[vc]
                if vc + EMB_BUFS < n_vc:
                    load_et(vc + EMB_BUFS, in_head=True)
                if use_voff:
                    nc.sync.dma_start(
                        out=voff_bc[:, :vn],
                        in_=voff_s[:, v0:v0 + vn].to_broadcast([P, vn]))
                nsl = (vn + 511) // 512
                for tp in range(TPN):
                    # pairs of 512-slices share one lout tile + one DMA out
                    for i0 in range(0, nsl, 2):
                        sls = [i for i in (i0, i0 + 1) if i < nsl]
                        ws = [min(512, vn - i * 512) for i in sls]
                        wtot = sum(ws)
                        pls = [psp.tile([P, 512], F32, tag="ps",
                                        name=f"plv_{vc}_{tp}_{i}")
                               for i in sls]
                        for kc in range(KH):
                            for j, i in enumerate(sls):
                                nc.tensor.matmul(
                                    pls[j][:, :ws[j]], hcT[:, kc, ts(tp, P)],
                                    et[:, kc, ds(i * 512, ws[j])],
                                    start=(kc == 0), stop=(kc == KH - 1))
                        lo = loutp.tile([P, 1024], BF16, tag="lo")
                        off = 0
                        for j in range(len(sls)):
                            evac_logits(lo[:, ds(off, ws[j])],
                                        pls[j][:, :ws[j]], tp)
                            off += ws[j]
                        if use_voff:
                            nc.vector.tensor_tensor(
                                out=lo[:, :wtot], in0=lo[:, :wtot],
                                in1=voff_bc[:, ds(i0 * 512, wtot)],
                                op=ALU.add)
                        nc.scalar.dma_start(
                            out=out_d[tp * P:(tp + 1) * P,
                                      v0 + i0 * 512:v0 + i0 * 512 + wtot],
                            in_=lo[:, :wtot])
    nc.compile()
    return nc


def host_prep(x, embed, W1, b1, g1, be1, W2, b2, g2, be2, W3, b3, gn, bn,
              n_steps=N_STEPS):
    """Pure-numpy input prep shared by all cores."""
    x = np.asarray(x).reshape(-1)
    embed = np.asarray(embed, dtype=np.float32)
    W1 = np.asarray(W1, dtype=np.float32)
    b1 = np.asarray(b1, dtype=np.float32)
    t_norm, _, _, A, _ = _step_consts(n_steps)
    h0 = embed[x]                                     # [T_total, HID]
    r1 = ((t_norm[:, None] * W1[HID][None, :] + b1[None, :])
          / A[:, None]).astype(ml_dtypes.bfloat16)[None]
    gnf = np.asarray(gn, dtype=np.float32)
    embt = np.ascontiguousarray(
        (embed * gnf[None, :]).T.astype(ml_dtypes.bfloat16))  # [HID, VOCAB]
    voff = (np.asarray(bn, dtype=np.float32) @ embed.T).astype(np.float32)
    return dict(
        h0=np.ascontiguousarray(h0),
        w1=np.ascontiguousarray(W1[:HID]).astype(ml_dtypes.bfloat16),
        r1=np.ascontiguousarray(r1),
        w2=np.asarray(W2, dtype=np.float32).astype(ml_dtypes.bfloat16),
        w3=np.asarray(W3, dtype=np.float32).astype(ml_dtypes.bfloat16),
        embt=embt,
        b2=np.asarray(b2, dtype=np.float32).astype(
            ml_dtypes.bfloat16).reshape(1, -1),
        b3=np.asarray(b3, dtype=np.float32).astype(
            ml_dtypes.bfloat16).reshape(1, -1),
        voff=voff.reshape(1, -1),
        g1=np.asarray(g1, dtype=np.float32),
        be1=np.asarray(be1, dtype=np.float32),
        g2=np.asarray(g2, dtype=np.float32),
        be2=np.asarray(be2, dtype=np.float32),
    )


_CACHE = {}


def _get_program(key, **kw):
    if key not in _CACHE:
        _CACHE[key] = build_program(**kw)
    return _CACHE[key]


def kernel(x, embed, W1, b1, g1, be1, W2, b2, g2, be2, W3, b3, gn, bn,
           run_kwargs=None):
    pre = host_prep(x, embed, W1, b1, g1, be1, W2, b2, g2, be2, W3, b3,
                    gn, bn)

    apply_gb1 = bool(np.any(pre["g1"] != 1.0) or np.any(pre["be1"] != 0.0))
    apply_gb2 = bool(np.any(pre["g2"] != 1.0) or np.any(pre["be2"] != 0.0))
    use_b2 = bool(np.any(np.asarray(b2)))
    use_b3 = bool(np.any(np.asarray(b3)))
    use_voff = bool(np.any(pre["voff"]))

    key = (apply_gb1, apply_gb2, use_b2, use_b3, use_voff)
    nc = _get_program(key, apply_gb1=apply_gb1, apply_gb2=apply_gb2,
                      use_b2=use_b2, use_b3=use_b3, use_voff=use_voff)

    common = {"w1": pre["w1"], "r1": pre["r1"], "w2": pre["w2"],
              "w3": pre["w3"], "embt": pre["embt"]}
    if use_b2:
        common["b2"] = pre["b2"]
    if use_b3:
        common["b3"] = pre["b3"]
    if use_voff:
        common["voff"] = pre["voff"]
    if apply_gb1 or apply_gb2:
        common["gb"] = np.stack([pre["g1"], pre["be1"], pre["g2"],
                                 pre["be2"]])

    in_maps = []
    for c in range(N_CORES):
        m = dict(common)
        m["h0t"] = np.ascontiguousarray(
            pre["h0"][c * T_CORE:(c + 1) * T_CORE].T)
        in_maps.append(m)

    res = bass_utils.run_bass_kernel_spmd(
        nc, in_maps, core_ids=list(range(N_CORES)), **(run_kwargs or {}))
    # device emits bf16 logits (halves the HBM write); upcast on host
    out = np.concatenate(
        [np.asarray(res.results[c]["logits"]).astype(np.float32)
         for c in range(N_CORES)], axis=0)
    kernel.last_results = res
    return out.reshape(B, S, VOCAB)



# revision 15
# speedup vs baseline: 1.0191x; 1.0191x over previous
"""Trainium2 Bass kernel for nn_DiffusionLM (dense_mlp).

Strategy (8 NeuronCores, data-parallel over tokens; 512 tokens/core):
  - Host: embedding gather + pre-transpose h0 -> h0T [HID, T_CORE] f32;
    weights cast to bf16; step-bias table r1[t] and step coefficient
    folded by the cumulative 1/sqrt(alpha) product A_t (LayerNorm is
    scale-invariant, so the per-step `isa` rescale of h is dropped and
    absorbed into r1/A_t and coef/A_t); vocab head (embed*gn).T in bf16.
  - Device diffusion (20 steps), all bf16 matmuls at 1 cycle/row:
      * h~T kept feature-major in SBUF (f32 master + bf16 copy), so the
        first-layer matmul needs no activation transpose.
      * z1/z2 LayerNorm+gelu fused into a single ACT pass per PSUM half:
        Gelu(psum*rstd + (-mu*rstd)) with per-partition scale/bias APs.
      * z1/z2 transposed back to feature-major by DMA XBAR transposes
        (SBUF->SBUF, off the PE).
      * h~ update is one gpsimd scalar_tensor_tensor: h~ += (-c~)*scoreT.
  - Final LN folded into the vocab projection: center h~T by mu (rank-0
    DMA broadcast of the mu row), fold rstd into the PSUM->SBUF evac.
  - Vocab projection streams embt bf16 from HBM (first chunks prefetched
    during diffusion), writes f32 logits.
"""

import numpy as np
import ml_dtypes

import concourse.bass as bass
import concourse.mybir as mybir
import concourse.tile as tile
from concourse import bacc, bass_utils
from concourse.bass import ds, ts
from concourse.masks import make_identity

dt = mybir.dt
F32 = dt.float32
F16 = dt.float16
BF16 = dt.bfloat16
I32 = dt.int32
AF = mybir.ActivationFunctionType
ALU = mybir.AluOpType

N_CORES = 8
VOCAB = 32000
HID = 512
DH = 2 * HID  # 1024
N_STEPS = 20
EPS = 1e-5
B, S = 2, 2048
T_TOTAL = B * S              # 4096
T_CORE = T_TOTAL // N_CORES  # 512
P = 128
TPN = T_CORE // P            # 4 token tiles
KH = HID // P                # 4
KD = DH // P                 # 8
RSQRT_MAGIC = 0x5F3759DF
VC = 2048                    # vocab stream chunk
N_PREF = 3                   # chunks prefetched during diffusion
EMB_BUFS = 4                 # embt chunk ring size


def _step_consts(n_steps):
    """Per-step scalars, ordered t = n_steps-1 .. 0, matching reference."""
    betas = np.linspace(0.0001, 0.02, n_steps, dtype=np.float32)
    alphas = (1.0 - betas).astype(np.float32)
    acp = np.cumprod(alphas, dtype=np.float32)
    tsx = np.arange(n_steps - 1, -1, -1)
    t_norm = (tsx.astype(np.float32) / np.float32(n_steps)).astype(np.float32)
    coef = (betas[tsx] / np.sqrt((np.float32(1.0) - acp[tsx]))).astype(np.float32)
    isa = (np.float32(1.0) / np.sqrt(alphas[tsx])).astype(np.float32)
    # A_s = prod_{j<s} isa_j; h = A*h~ and LN() erases the final A.
    A = np.ones(n_steps, dtype=np.float64)
    for s_ in range(1, n_steps):
        A[s_] = A[s_ - 1] * isa[s_ - 1]
    ctil = (coef.astype(np.float64) / A).astype(np.float32)
    return t_norm, coef, isa, A.astype(np.float32), ctil


def build_program(n_steps=N_STEPS, vocab=VOCAB,
                  apply_gb1=False, apply_gb2=False,
                  use_b2=False, use_b3=False, use_voff=False):
    nc = bacc.Bacc("TRN2", target_bir_lowering=False, debug=False,
                   num_devices=N_CORES)

    h0t_d = nc.dram_tensor("h0t", [HID, T_CORE], F32, kind="ExternalInput").ap()
    w1_d = nc.dram_tensor("w1", [HID, DH], BF16, kind="ExternalInput").ap()
    r1_d = nc.dram_tensor("r1", [1, n_steps, DH], BF16,
                          kind="ExternalInput").ap()
    w2_d = nc.dram_tensor("w2", [DH, DH], BF16, kind="ExternalInput").ap()
    w3_d = nc.dram_tensor("w3", [DH, HID], BF16, kind="ExternalInput").ap()
    emb_d = nc.dram_tensor("embt", [HID, vocab], BF16,
                           kind="ExternalInput").ap()
    out_d = nc.dram_tensor("logits", [T_CORE, vocab], F16,
                           kind="ExternalOutput").ap()
    b2_d = b3_d = voff_d = gb_d = None
    if use_b2:
        b2_d = nc.dram_tensor("b2", [1, DH], BF16, kind="ExternalInput").ap()
    if use_b3:
        b3_d = nc.dram_tensor("b3", [1, HID], BF16, kind="ExternalInput").ap()
    if use_voff:
        voff_d = nc.dram_tensor("voff", [1, vocab], F32,
                                kind="ExternalInput").ap()
    if apply_gb1 or apply_gb2:
        gb_d = nc.dram_tensor("gb", [4, DH], F32, kind="ExternalInput").ap()

    _, _, isa_c, A_c, ctil = _step_consts(n_steps)
    eps1 = (EPS / (A_c.astype(np.float64) ** 2)).astype(np.float32)
    a_fin = float(A_c[-1] * isa_c[-1])
    eps_fin = float(EPS / (a_fin * a_fin))

    with tile.TileContext(nc) as tc:
      with (
          tc.tile_pool(name="wpool", bufs=1) as wpool,
          tc.tile_pool(name="work", bufs=3) as work,
          tc.tile_pool(name="emb", bufs=EMB_BUFS) as embp,
          tc.tile_pool(name="lout", bufs=3) as loutp,
          tc.tile_pool(name="ps", bufs=8, space="PSUM") as psp,
      ):
            # ---- resident constants / weights ----
            ones1 = wpool.tile([1, P], BF16)
            nc.vector.memset(ones1, 1.0)
            ident = wpool.tile([P, P], F32)
            make_identity(nc, ident)

            # startup loads split across both HWDGE queues (sync+scalar) so
            # the first-step matmuls aren't gated on one serial queue.
            hT = wpool.tile([P, KH, T_CORE], F32)
            h0t_r = h0t_d.rearrange("(k p) t -> p k t", p=P)
            w1_r = w1_d.rearrange("(k p) n -> p k n", p=P)
            w1a = wpool.tile([P, KH, DH], BF16)
            for kc in range(KH):
                eng = nc.sync if kc % 2 == 0 else nc.scalar
                eng.dma_start(out=hT[:, kc, :], in_=h0t_r[:, kc, :])
            for kc in range(KH):
                eng = nc.sync if kc % 2 == 1 else nc.scalar
                eng.dma_start(out=w1a[:, kc, :], in_=w1_r[:, kc, :])
            w2_r = w2_d.rearrange("(k p) n -> p k n", p=P)
            w2a = wpool.tile([P, KD, DH], BF16)
            for half in range(2):
                eng = nc.sync if half == 0 else nc.scalar
                eng.dma_start(out=w2a[:, ds(half * 4, 4), :],
                              in_=w2_r[:, ds(half * 4, 4), :])
            w3a = wpool.tile([P, KD, HID], BF16)
            nc.gpsimd.dma_start(out=w3a,
                                in_=w3_d.rearrange("(k p) n -> p k n", p=P))
            w1s = [w1a[:, kc, :] for kc in range(KH)]
            w2s = [w2a[:, kc, :] for kc in range(KD)]
            w3s = [w3a[:, kc, :] for kc in range(KD)]

            b2s = b3s = voff_s = onesrow = gbs = None
            if use_b2:
                b2s = wpool.tile([1, DH], BF16)
                nc.sync.dma_start(out=b2s, in_=b2_d)
            if use_b3:
                b3s = wpool.tile([1, HID], BF16)
                nc.sync.dma_start(out=b3s, in_=b3_d)
                onesrow = wpool.tile([1, T_CORE], BF16)
                nc.vector.memset(onesrow, 1.0)
            if use_voff:
                voff_s = wpool.tile([1, vocab], F32)
                nc.sync.dma_start(out=voff_s, in_=voff_d)
                voff_bc = wpool.tile([P, VC], F32)
            if gb_d is not None:
                gbs = wpool.tile([P, 4, DH], F32)
                nc.sync.dma_start(out=gbs, in_=gb_d.to_broadcast([P, 4, DH]))

            # persistent h~T (feature-major): f32 master + bf16 matmul copy
            # (initial casts on DVE: the ACT queue is busy loading w2/w3)
            hbf = wpool.tile([P, KH, T_CORE], BF16)
            for kc in range(KH):
                nc.vector.tensor_copy(out=hbf[:, kc, :], in_=hT[:, kc, :])
            hcT = wpool.tile([P, KH, T_CORE], BF16)
            magict = wpool.tile([P, TPN], I32)
            nc.vector.memset(magict, RSQRT_MAGIC)

            def rsqrt_chain(mvp, n, eps):
                """DVE chain on [P,n]: returns (rstd, negbias) tiles.

                mvp is [P,n,2] f32 (mean, var) from bn_aggr; eps is the
                A-rescaled epsilon keeping LN scale-exact vs the reference."""
                u = work.tile([P, n], F32, tag="u", bufs=4)
                yv = work.tile([P, n], F32, tag="yv", bufs=4)
                t2 = work.tile([P, n], F32, tag="t2", bufs=4)
                nb = work.tile([P, n], F32, tag="nb", bufs=4)
                nc.vector.tensor_scalar(out=u, in0=mvp[:, :, 1], scalar1=eps,
                                        scalar2=None, op0=ALU.add)
                nc.vector.tensor_scalar(out=t2.bitcast(I32),
                                        in0=u.bitcast(I32), scalar1=1,
                                        scalar2=None,
                                        op0=ALU.logical_shift_right)
                nc.vector.tensor_tensor(out=yv.bitcast(I32),
                                        in0=magict[:, :n],
                                        in1=t2.bitcast(I32), op=ALU.subtract)
                # Newton 1: rstd = est * (1.5 - 0.5*u*est^2), fused
                nc.vector.tensor_tensor(out=t2, in0=yv, in1=yv, op=ALU.mult)
                nc.vector.scalar_tensor_tensor(out=t2, in0=t2, scalar=-0.5,
                                               in1=u, op0=ALU.mult,
                                               op1=ALU.mult)
                nc.vector.scalar_tensor_tensor(out=yv, in0=t2, scalar=1.5,
                                               in1=yv, op0=ALU.add,
                                               op1=ALU.mult)
                # nb = -mean * rstd
                nc.vector.scalar_tensor_tensor(out=nb, in0=mvp[:, :, 0],
                                               scalar=-1.0, in1=yv,
                                               op0=ALU.mult, op1=ALU.mult)
                return yv, nb

            # LN-chain emission groups: tile 0 alone (unblocks ACT/PE
            # early), then (1,2), then 3.
            LN_GROUPS = {0: (0,), 2: (1, 2), 3: (3,)}

            def ln_gelu_group(pps, sts, group, zall, gb_idx, eps):
                """Stats-chain + fused LN/gelu into zall[:, t, :]."""
                ng = len(group)
                mvp = work.tile([P, ng, 2], F32, tag=f"mv{ng}", bufs=4)
                for i, t in enumerate(group):
                    nc.vector.bn_aggr(out=mvp[:, i, :], in_=sts[t])
                rstd, nb = rsqrt_chain(mvp, ng, eps)
                for i, t in enumerate(group):
                    if gb_idx is None:
                        for h in range(2):
                            nc.scalar.activation(
                                out=zall[:, t, ds(h * 512, 512)],
                                in_=pps[t][h], func=AF.Gelu,
                                scale=rstd[:, i:i + 1], bias=nb[:, i:i + 1])
                    else:
                        # general path: g/be per-feature after LN
                        zf = work.tile([P, DH], F32, tag="zf", bufs=2)
                        for h in range(2):
                            nc.vector.tensor_scalar(
                                out=zf[:, ds(h * 512, 512)], in0=pps[t][h],
                                scalar1=mvp[:, i, 0:1],
                                scalar2=rstd[:, i:i + 1],
                                op0=ALU.subtract, op1=ALU.mult)
                        g_t = gbs[:, gb_idx, :]
                        be_t = gbs[:, gb_idx + 1, :]
                        nc.vector.tensor_tensor(out=zf, in0=zf, in1=g_t,
                                                op=ALU.mult)
                        nc.vector.tensor_tensor(out=zf, in0=zf, in1=be_t,
                                                op=ALU.add)
                        nc.scalar.activation(out=zall[:, t, :], in_=zf,
                                             func=AF.Gelu)

            # ================= diffusion =================
            ets = []

            def load_et(vc, in_head=False):
                v0e = vc * VC
                vne = min(VC, vocab - v0e)
                et = embp.tile([P, KH, VC], BF16, tag="et",
                               name=f"et_{vc}")
                # during diffusion the HWDGE queues are busy with
                # transposes; in the head they are free, so alternate.
                eng = (nc.sync if vc % 2 == 0 else nc.gpsimd) if in_head \
                    else nc.gpsimd
                for kc in range(KH):
                    eng.dma_start(
                        out=et[:, kc, :vne],
                        in_=emb_d[kc * P:(kc + 1) * P, v0e:v0e + vne])
                ets.append(et)
            for step in range(n_steps):
                cneg = -float(ctil[step])
                r1row = work.tile([1, DH], BF16, tag="r1row", bufs=2,
                                  name=f"r1row_{step}")
                nc.gpsimd.dma_start(out=r1row, in_=r1_d[:, step, :])

                # ---- layer 1: z1 = gelu(LN(h~ @ W1 + r1~)) ----
                z1ps, z1st = {}, {}
                z1all = work.tile([P, TPN, DH], BF16, tag="z1all", bufs=2,
                                  name=f"z1all_{step}")
                z1T = work.tile([P, TPN, KD, P], BF16, tag="z1T", bufs=2,
                                name=f"z1T_{step}")
                for tp in range(TPN):
                    pp = [psp.tile([P, 512], F32, tag="ps",
                                   name=f"ps1_{step}_{tp}_{h}")
                          for h in range(2)]
                    for kc in range(KH):
                        for h in range(2):
                            nc.tensor.matmul(pp[h], hbf[:, kc, ts(tp, P)],
                                             w1s[kc][:, ds(h * 512, 512)],
                                             start=(kc == 0), stop=False)
                    for h in range(2):
                        nc.tensor.matmul(pp[h], ones1,
                                         r1row[:, ds(h * 512, 512)],
                                         start=False, stop=True)
                    st = work.tile([P, 2, 6], F32, tag="st", bufs=4)
                    for h in range(2):
                        nc.vector.bn_stats(out=st[:, h, :], in_=pp[h])
                    z1ps[tp], z1st[tp] = pp, st
                    if tp in LN_GROUPS:
                        ln_gelu_group(z1ps, z1st, LN_GROUPS[tp], z1all,
                                      0 if apply_gb1 else None,
                                      float(eps1[step]))
                    if tp in LN_GROUPS:
                        for t in LN_GROUPS[tp]:
                            for h in range(2):
                                eng = nc.sync if (t + h) % 2 == 0 else nc.scalar
                                eng.dma_start(
                                    out=z1T[:, t, ds(h * KH, KH), :],
                                    in_=z1all[:, t, ds(h * 512, 512)],
                                    transpose=True)


                # ---- layer 2: z2 = gelu(LN(z1 @ W2 (+b2))) ----
                z2ps, z2st = {}, {}
                z2all = work.tile([P, TPN, DH], BF16, tag="z2all", bufs=2,
                                  name=f"z2all_{step}")
                z2T = work.tile([P, TPN, KD, P], BF16, tag="z2T", bufs=2,
                                name=f"z2T_{step}")
                for tp in range(TPN):
                    pp = [psp.tile([P, 512], F32, tag="ps",
                                   name=f"ps2_{step}_{tp}_{h}")
                          for h in range(2)]
                    for kc in range(KD):
                        for h in range(2):
                            nc.tensor.matmul(pp[h], z1T[:, tp, kc, :],
                                             w2s[kc][:, ds(h * 512, 512)],
                                             start=(kc == 0),
                                             stop=(kc == KD - 1 and not use_b2))
                    if use_b2:
                        for h in range(2):
                            nc.tensor.matmul(pp[h], ones1,
                                             b2s[:, ds(h * 512, 512)],
                                             start=False, stop=True)
                    st = work.tile([P, 2, 6], F32, tag="st", bufs=4)
                    for h in range(2):
                        nc.vector.bn_stats(out=st[:, h, :], in_=pp[h])
                    z2ps[tp], z2st[tp] = pp, st
                    if tp in LN_GROUPS:
                        ln_gelu_group(z2ps, z2st, LN_GROUPS[tp], z2all,
                                      2 if apply_gb2 else None,
                                      float(eps1[step]))
                    if tp in LN_GROUPS:
                        for t in LN_GROUPS[tp]:
                            for h in range(2):
                                eng = nc.sync if (t + h) % 2 == 1 else nc.scalar
                                eng.dma_start(
                                    out=z2T[:, t, ds(h * KH, KH), :],
                                    in_=z2all[:, t, ds(h * 512, 512)],
                                    transpose=True)


                # ---- layer 3 (feature-major) + h~ update ----
                # hbf for the next step is produced directly from PSUM per
                # token-half so the next mm1 isn't gated on the f32 master.
                ps3 = [psp.tile([P, 512], F32, tag="ps",
                                name=f"ps3_{step}_{mc}")
                       for mc in range(KH)]
                for hn in range(2):  # token halves so PE starts earlier
                    sl = ds(hn * 256, 256)
                    for mc in range(KH):
                        for kc in range(KD):
                            nc.tensor.matmul(
                                ps3[mc][:, sl], w3s[kc][:, ts(mc, P)],
                                z2T[:, 2 * hn:2 * hn + 2, kc, :],
                                start=(kc == 0),
                                stop=(kc == KD - 1 and not use_b3))
                        if use_b3:
                            nc.tensor.matmul(ps3[mc][:, sl],
                                             b3s[:, ts(mc, P)], onesrow[:, sl],
                                             start=False, stop=True)
                    if step < n_steps - 1:
                        for mc in range(KH):
                            nc.vector.scalar_tensor_tensor(
                                out=hbf[:, mc, sl], in0=ps3[mc][:, sl],
                                scalar=cneg, in1=hT[:, mc, sl],
                                op0=ALU.mult, op1=ALU.add)
                # f32 master update off the DVE: stage score via ACT copy,
                # then gpsimd does hT += cneg*score (SBUF-only operands).
                for mc in range(KH):
                    nc.vector.scalar_tensor_tensor(
                        out=hT[:, mc, :], in0=ps3[mc], scalar=cneg,
                        in1=hT[:, mc, :], op0=ALU.mult, op1=ALU.add)

                # prefetch first embt chunks late in diffusion
                if n_steps - 1 - N_PREF <= step < n_steps - 1:
                    load_et(step - (n_steps - 1 - N_PREF))

            # ============ final LN (folded into vocab head) ============
            # PE transposes h~T into token-major PSUM tiles for stats
            # (the PE is otherwise idle between diffusion and vocab).
            pst = [psp.tile([P, 512], F32, tag="ps", name=f"pst_{ti}")
                   for ti in range(TPN)]
            mvf = wpool.tile([P, TPN, 2], F32)
            for ti in range(TPN):
                for kc in range(KH):
                    nc.tensor.transpose(pst[ti][:, ts(kc, P)],
                                        hT[:, kc, ts(ti, P)], ident)
                stf = work.tile([P, KH, 6], F32, tag="stf", bufs=4)
                for kc in range(KH):
                    nc.vector.bn_stats(out=stf[:, kc, :],
                                       in_=pst[ti][:, ts(kc, P)])
                nc.vector.bn_aggr(out=mvf[:, ti, :], in_=stf)
            rsf, _nbf = rsqrt_chain(mvf, TPN, eps_fin)
            # mu row -> [1, T_CORE] bf16 via PE transpose + one DMA, then
            # broadcast across partitions with a rank-1 PE matmul.
            ptm = psp.tile([P, 512], F32, tag="ps")
            nc.tensor.transpose(ptm[0:TPN, 0:P], mvf[:, :, 0], ident)
            mur4 = wpool.tile([P, P], BF16, name="mur4")
            nc.vector.tensor_copy(out=mur4[0:TPN, :], in_=ptm[0:TPN, 0:P])
            murow = wpool.tile([1, T_CORE], BF16, name="murow")
            nc.sync.dma_start(out=murow, in_=mur4[0:TPN, :])
            mu_bc = psp.tile([P, 512], F32, tag="ps", name="mu_bc")
            nc.tensor.matmul(mu_bc, ones1, murow, start=True, stop=True)
            for kc in range(KH):
                nc.vector.tensor_tensor(out=hcT[:, kc, :], in0=hT[:, kc, :],
                                        in1=mu_bc, op=ALU.subtract)

            # ================= vocab head =================
            n_vc = (vocab + VC - 1) // VC

            def evac_logits(lo_sl, pl_sl, tp):
                nc.vector.tensor_scalar(out=lo_sl, in0=pl_sl,
                                        scalar1=rsf[:, tp:tp + 1],
                                        scalar2=None, op0=ALU.mult)

            for vc in range(len(ets), min(EMB_BUFS, n_vc)):
                load_et(vc, in_head=True)
            for vc in range(n_vc):
                v0 = vc * VC
                vn = min(VC, vocab - v0)
                et = ets[vc]
                if vc + EMB_BUFS < n_vc:
                    load_et(vc + EMB_BUFS, in_head=True)
                if use_voff:
                    nc.sync.dma_start(
                        out=voff_bc[:, :vn],
                        in_=voff_s[:, v0:v0 + vn].to_broadcast([P, vn]))
                nsl = (vn + 511) // 512
                for tp in range(TPN):
                    # pairs of 512-slices share one lout tile + one DMA out
                    for i0 in range(0, nsl, 2):
                        sls = [i for i in (i0, i0 + 1) if i < nsl]
                        ws = [min(512, vn - i * 512) for i in sls]
                        wtot = sum(ws)
                        pls = [psp.tile([P, 512], F32, tag="ps",
                                        name=f"plv_{vc}_{tp}_{i}")
                               for i in sls]
                        for kc in range(KH):
                            for j, i in enumerate(sls):
                                nc.tensor.matmul(
                                    pls[j][:, :ws[j]], hcT[:, kc, ts(tp, P)],
                                    et[:, kc, ds(i * 512, ws[j])],
                                    start=(kc == 0), stop=(kc == KH - 1))
                        lo = loutp.tile([P, 1024], F16, tag="lo")
                        off = 0
                        for j in range(len(sls)):
                            evac_logits(lo[:, ds(off, ws[j])],
                                        pls[j][:, :ws[j]], tp)
                            off += ws[j]
                        if use_voff:
                            nc.vector.tensor_tensor(
                                out=lo[:, :wtot], in0=lo[:, :wtot],
                                in1=voff_bc[:, ds(i0 * 512, wtot)],
                                op=ALU.add)
                        nc.scalar.dma_start(
                            out=out_d[tp * P:(tp + 1) * P,
                                      v0 + i0 * 512:v0 + i0 * 512 + wtot],
                            in_=lo[:, :wtot])
    nc.compile()
    return nc


def host_prep(x, embed, W1, b1, g1, be1, W2, b2, g2, be2, W3, b3, gn, bn,
              n_steps=N_STEPS):
    """Pure-numpy input prep shared by all cores."""
    x = np.asarray(x).reshape(-1)
    embed = np.asarray(embed, dtype=np.float32)
    W1 = np.asarray(W1, dtype=np.float32)
    b1 = np.asarray(b1, dtype=np.float32)
    t_norm, _, _, A, _ = _step_consts(n_steps)
    h0 = embed[x]                                     # [T_total, HID]
    r1 = ((t_norm[:, None] * W1[HID][None, :] + b1[None, :])
          / A[:, None]).astype(ml_dtypes.bfloat16)[None]
    gnf = np.asarray(gn, dtype=np.float32)
    embt = np.ascontiguousarray(
        (embed * gnf[None, :]).T.astype(ml_dtypes.bfloat16))  # [HID, VOCAB]
    voff = (np.asarray(bn, dtype=np.float32) @ embed.T).astype(np.float32)
    return dict(
        h0=np.ascontiguousarray(h0),
        w1=np.ascontiguousarray(W1[:HID]).astype(ml_dtypes.bfloat16),
        r1=np.ascontiguousarray(r1),
        w2=np.asarray(W2, dtype=np.float32).astype(ml_dtypes.bfloat16),
        w3=np.asarray(W3, dtype=np.float32).astype(ml_dtypes.bfloat16),
        embt=embt,
        b2=np.asarray(b2, dtype=np.float32).astype(
            ml_dtypes.bfloat16).reshape(1, -1),
        b3=np.asarray(b3, dtype=np.float32).astype(
            ml_dtypes.bfloat16).reshape(1, -1),
        voff=voff.reshape(1, -1),
        g1=np.asarray(g1, dtype=np.float32),
        be1=np.asarray(be1, dtype=np.float32),
        g2=np.asarray(g2, dtype=np.float32),
        be2=np.asarray(be2, dtype=np.float32),
    )


_CACHE = {}


def _get_program(key, **kw):
    if key not in _CACHE:
        _CACHE[key] = build_program(**kw)
    return _CACHE[key]


def kernel(x, embed, W1, b1, g1, be1, W2, b2, g2, be2, W3, b3, gn, bn,
           run_kwargs=None):
    pre = host_prep(x, embed, W1, b1, g1, be1, W2, b2, g2, be2, W3, b3,
                    gn, bn)

    apply_gb1 = bool(np.any(pre["g1"] != 1.0) or np.any(pre["be1"] != 0.0))
    apply_gb2 = bool(np.any(pre["g2"] != 1.0) or np.any(pre["be2"] != 0.0))
    use_b2 = bool(np.any(np.asarray(b2)))
    use_b3 = bool(np.any(np.asarray(b3)))
    use_voff = bool(np.any(pre["voff"]))

    key = (apply_gb1, apply_gb2, use_b2, use_b3, use_voff)
    nc = _get_program(key, apply_gb1=apply_gb1, apply_gb2=apply_gb2,
                      use_b2=use_b2, use_b3=use_b3, use_voff=use_voff)

    common = {"w1": pre["w1"], "r1": pre["r1"], "w2": pre["w2"],
              "w3": pre["w3"], "embt": pre["embt"]}
    if use_b2:
        common["b2"] = pre["b2"]
    if use_b3:
        common["b3"] = pre["b3"]
    if use_voff:
        common["voff"] = pre["voff"]
    if apply_gb1 or apply_gb2:
        common["gb"] = np.stack([pre["g1"], pre["be1"], pre["g2"],
                                 pre["be2"]])

    in_maps = []
    for c in range(N_CORES):
        m = dict(common)
        m["h0t"] = np.ascontiguousarray(
            pre["h0"][c * T_CORE:(c + 1) * T_CORE].T)
        in_maps.append(m)

    res = bass_utils.run_bass_kernel_spmd(
        nc, in_maps, core_ids=list(range(N_CORES)), **(run_kwargs or {}))
    # device emits fp16 logits (halves the HBM write); upcast on host
    out = np.concatenate(
        [np.asarray(res.results[c]["logits"]).astype(np.float32)
         for c in range(N_CORES)], axis=0)
    kernel.last_results = res
    return out.reshape(B, S, VOCAB)

